# revision 22
# baseline (speedup 1.0000x reference)
"""Trainium2 Bass kernel for nn_CrossAttention (seq_len==1 cross attention,
dual-stream transformer block pair).

Math notes (exact simplifications, valid for any input values):
  - Both attentions have seq_len==1 for q and kv, so softmax over the single
    kv position is exactly 1.0 and attention output == V projection:
        mha(q_in, kv_in) = (kv_in @ wv.T + bv) @ out_w.T + out_b
    The q/k projections are dead code.  Folding the two matmuls:
        attn = kv_in @ (out_w @ wv).T + (out_w @ bv + out_b)
  - LayerNorm affine (g, b) of ln1/ln2 is folded into the following FFN
    weights host-side; residual-path affine and biases are applied on-device
    only when they are non-trivial (they are zeros/ones for the reference
    setup_inputs, so the fast path emits no extra instructions).
  - LayerNorm is scale invariant: LN(c*x) == LN(x).  This lets per-tensor
    pow2 scales (used by the fp8 path) cancel without any rescale ops.

v4 layout: host pre-transposes dna/mol (and pre-permutes all tile layouts)
so every SBUF tile is filled by ONE straight DMA — no xbar DMA transposes.
Identity/constant tiles are created BEFORE any DMA issue so the PE warmup
stream starts immediately instead of queueing behind the weight preload.
Queues: sync = transposed x, gpsimd = straight x, scalar = weights + out.
Per-tile PE stream is software-pipelined one tile deep (FFN2 of tile t-1
runs between attn and zT/FFN1 of tile t).  LN sqrt/recip batched per group.
"""

import numpy as np
import ml_dtypes
from contextlib import ExitStack

import concourse.bass as bass
import concourse.tile as tile
from concourse import bacc, mybir
from concourse.bass_utils import run_bass_kernel_spmd

E = 512
HID = 1024
NCORES = 8
EPS = 1e-5
P = 128

BF16 = mybir.dt.bfloat16
F32 = mybir.dt.float32
FP8 = mybir.dt.float8e4
BF = ml_dtypes.bfloat16
F8 = ml_dtypes.float8_e4m3

# fp8 path: pow2 per-tensor scales, all cancelled by LN scale invariance.
#   straight x: bf16(64*x)          transposed x: fp8(x)
#   W,U,V: fp8(64*W)                => attn psum scale 64, h scale 64
#   z written at scale 128 (inv folded); zT = fp8(z/64) at scale 2
#   gt = fp8(relu(psum1)/64) at scale 2; ffn2 psum scale 2*64 = 128 == z
USE_FP8 = True

_prog_cache = {}


def _build_program(rows_per_core: int, rmacro: int, flags: tuple,
                   fp8: bool):
    """Build + compile the per-core Bass program.

    flags = (use_c0, use_c1, use_d0, use_d1, use_e0, use_e1,
             aff_a0, aff_a1, aff_b0, aff_b1)
    """
    (use_c0, use_c1, use_d0, use_d1, use_e0, use_e1,
     aff_a0, aff_a1, aff_b0, aff_b1) = flags
    use_c = (use_c0, use_c1)
    use_d = (use_d0, use_d1)
    use_e = (use_e0, use_e1)
    aff_a = (aff_a0, aff_a1)
    aff_b = (aff_b0, aff_b1)

    R = rmacro
    NT = rows_per_core // R
    RC = R // P
    KE = E // P    # 4 K-chunks over E
    KH = HID // P  # 8 K-chunks over HID
    XDT = FP8 if fp8 else BF16
    DR = mybir.MatmulPerfMode.DoubleRow if fp8 else None
    KSTEP = 2 if fp8 else 1

    nc = bacc.Bacc("TRN2", target_bir_lowering=False, debug=False,
                   num_devices=NCORES)

    # straight x: xs[mt, p, rc, e] = x[mt*R + rc*P + p, e]
    # transposed x: xt[mt, p, c, r] = x[mt*R + r, c*P + p]
    xs_d = {}
    xt_d = {}
    for s, nm in ((0, "dna"), (1, "mol")):
        xs_d[s] = nc.dram_tensor(f"xs_{nm}", [NT, P, RC, E], BF16,
                                 kind="ExternalInput").ap()
        xt_d[s] = nc.dram_tensor(f"xt_{nm}", [NT, P, KE, R], XDT,
                                 kind="ExternalInput").ap()
    # out[mt, s, p, rc, e] = result[mt*R + rc*P + p, s*E + e]
    out = nc.dram_tensor("out", [NT, 2, P, RC, E], BF16,
                         kind="ExternalOutput").ap()

    wts = {}
    for s in range(2):
        wts[f"w{s}"] = nc.dram_tensor(f"w{s}", [P, KE, E], XDT,
                                      kind="ExternalInput").ap()
        wts[f"u{s}"] = nc.dram_tensor(f"u{s}", [P, KE, HID], XDT,
                                      kind="ExternalInput").ap()
        wts[f"v{s}"] = nc.dram_tensor(f"v{s}", [P, KH, E], XDT,
                                      kind="ExternalInput").ap()
        if use_c[s]:
            wts[f"c{s}"] = nc.dram_tensor(f"c{s}", [1, E], BF16,
                                          kind="ExternalInput").ap()
        if use_d[s]:
            wts[f"d{s}"] = nc.dram_tensor(f"d{s}", [1, HID], BF16,
                                          kind="ExternalInput").ap()
        if use_e[s]:
            wts[f"e{s}"] = nc.dram_tensor(f"e{s}", [1, E], BF16,
                                          kind="ExternalInput").ap()
        if aff_a[s]:
            wts[f"ga{s}"] = nc.dram_tensor(f"ga{s}", [1, E], BF16,
                                           kind="ExternalInput").ap()
            wts[f"ba{s}"] = nc.dram_tensor(f"ba{s}", [1, E], BF16,
                                           kind="ExternalInput").ap()
        if aff_b[s]:
            wts[f"gb{s}"] = nc.dram_tensor(f"gb{s}", [1, E], F32,
                                           kind="ExternalInput").ap()
            wts[f"bb{s}"] = nc.dram_tensor(f"bb{s}", [1, E], F32,
                                           kind="ExternalInput").ap()

    SUB = mybir.AluOpType.subtract
    MULT = mybir.AluOpType.mult
    Relu = mybir.ActivationFunctionType.Relu
    Sqrt = mybir.ActivationFunctionType.Sqrt
    Copy = mybir.ActivationFunctionType.Copy

    with tile.TileContext(nc) as tc:
        with ExitStack() as ctx:
            const = ctx.enter_context(tc.tile_pool(name="const", bufs=1))
            xin = ctx.enter_context(tc.tile_pool(name="xin", bufs=4))
            xtp = ctx.enter_context(tc.tile_pool(name="xtp", bufs=4))
            hpre = ctx.enter_context(tc.tile_pool(name="hpre", bufs=10))
            zpool = ctx.enter_context(tc.tile_pool(name="zpool", bufs=6))
            ztp = ctx.enter_context(tc.tile_pool(name="ztp", bufs=4))
            gpool = ctx.enter_context(tc.tile_pool(name="gpool", bufs=4))
            ypool = ctx.enter_context(tc.tile_pool(name="ypool", bufs=10))
            opool = ctx.enter_context(tc.tile_pool(name="opool", bufs=4))
            stats = ctx.enter_context(tc.tile_pool(name="stats", bufs=40))
            psA = ctx.enter_context(
                tc.tile_pool(name="psA", bufs=4, space="PSUM"))
            psG = ctx.enter_context(
                tc.tile_pool(name="psG", bufs=2, space="PSUM"))
            psT = ctx.enter_context(
                tc.tile_pool(name="psT", bufs=2, space="PSUM"))

            # --- consts FIRST: PE warmup must not wait on the DMA preload ---
            ident = const.tile([P, P], BF16, tag="ident")
            from concourse.masks import make_identity
            make_identity(nc, ident[...])
            # LN sqrt args (fp8: h at scale 64, y at scale 128):
            #   LN1: 1/inv = sqrt(var_h/16384 + eps/4)   -> z at scale 128
            #   LN2: 1/inv = sqrt(var_y + 16384*eps)     -> out at scale 1
            if fp8:
                eps_a, sc_a = EPS / 4.0, 1.0 / 16384.0
                eps_b, sc_b = EPS * 16384.0, 1.0
            else:
                eps_a, sc_a = EPS, 1.0
                eps_b, sc_b = EPS, 1.0
            eps_sb_a = const.tile([P, 1], F32, tag="eps_a")
            nc.vector.memset(eps_sb_a[...], eps_a)
            eps_sb_b = const.tile([P, 1], F32, tag="eps_b")
            nc.vector.memset(eps_sb_b[...], eps_b)
            ones_sb = const.tile([1, R], BF16, tag="ones")
            nc.vector.memset(ones_sb[...], 1.0)

            w_sb = {}
            for name, ap in wts.items():
                t = const.tile(list(ap.shape), ap.dtype, tag=f"w_{name}")
                w_sb[name] = t

            def loads(mt):
                """issue straight + transposed loads of both streams.

                sync: transposed x (mol first: attn s0 needs molT).
                gpsimd: straight x (dna first: residual s0 consumes it first).
                """
                xt_m = xtp.tile([P, KE, R], XDT, tag="xT")
                xt_dd = xtp.tile([P, KE, R], XDT, tag="xT")
                nc.sync.dma_start(out=xt_m[...], in_=xt_d[1][mt])
                nc.sync.dma_start(out=xt_dd[...], in_=xt_d[0][mt])
                x_dd = xin.tile([P, RC, E], BF16, tag="xin")
                x_m = xin.tile([P, RC, E], BF16, tag="xin")
                nc.gpsimd.dma_start(out=x_dd[...], in_=xs_d[0][mt])
                nc.gpsimd.dma_start(out=x_m[...], in_=xs_d[1][mt])
                return [x_dd, x_m], [xt_dd, xt_m]

            # --- startup issue schedule (per-queue ~85 GB/s, ~12us/MB) ---
            # PE needs (us, approx): w0@12 w1@15 u0@19 u1@26 v0@48 v1@55.
            nc.scalar.dma_start(out=w_sb["w0"][...], in_=wts["w0"])
            x_s, xT_s = loads(0)
            nc.scalar.dma_start(out=w_sb["w1"][...], in_=wts["w1"])
            nc.scalar.dma_start(out=w_sb["u0"][:, :2, :],
                                in_=wts["u0"][:, :2, :])
            nc.sync.dma_start(out=w_sb["u0"][:, 2:, :],
                              in_=wts["u0"][:, 2:, :])
            nc.gpsimd.dma_start(out=w_sb["u1"][:, 2:, :],
                                in_=wts["u1"][:, 2:, :])
            nc.sync.dma_start(out=w_sb["u1"][:, :2, :],
                              in_=wts["u1"][:, :2, :])
            nc.scalar.dma_start(out=w_sb["v0"][...], in_=wts["v0"])
            nc.gpsimd.dma_start(out=w_sb["v1"][...], in_=wts["v1"])
            for name in wts:
                if name[0] not in "wuv":
                    nc.scalar.dma_start(out=w_sb[name][...], in_=wts[name])

            # replicated affine tiles (only when needed)
            rep = {}
            for s in range(2):
                if aff_a[s]:
                    for nm in (f"ga{s}", f"ba{s}"):
                        r = const.tile([P, E], BF16, tag=f"rep_{nm}")
                        nc.sync.dma_start(out=r[...],
                                          in_=wts[nm].to_broadcast((P, E)))
                        rep[nm] = r
                if aff_b[s]:
                    for nm in (f"gb{s}", f"bb{s}"):
                        r = const.tile([P, E], F32, tag=f"rep_{nm}")
                        nc.sync.dma_start(out=r[...],
                                          in_=wts[nm].to_broadcast((P, E)))
                        rep[nm] = r

            def ln_group(srcs, dsts, eps_sb, sq_scale):
                """batched LN over RC chunks: one sqrt+recip for the group."""
                mv = stats.tile([P, RC, 2], F32, tag="mv")
                for rc in range(RC):
                    st6 = stats.tile([P, 6], F32, tag="st6")
                    nc.vector.bn_stats(out=st6[...], in_=srcs[rc])
                    nc.vector.bn_aggr(out=mv[:, rc, :], in_=st6[...])
                inv = stats.tile([P, RC, 1], F32, tag="inv")
                nc.scalar.activation(out=inv[...], in_=mv[:, :, 1:2],
                                     func=Sqrt, bias=eps_sb[...],
                                     scale=sq_scale)
                nc.vector.reciprocal(out=inv[...], in_=inv[...])
                for rc in range(RC):
                    nc.vector.tensor_scalar(
                        out=dsts[rc], in0=srcs[rc],
                        scalar1=mv[:, rc, 0:1], scalar2=inv[:, rc, 0:1],
                        op0=SUB, op1=MULT)

            def attn_ln1(s, x_s, xT_s):
                """attn + LN1 -> z (bf16), h1 (residual input for LN2)."""
                x = x_s[s]
                kvT = xT_s[1 - s]
                z = zpool.tile([P, RC, E], BF16, tag="z")
                hps = []
                for rc in range(RC):
                    ps = psA.tile([P, E], F32, tag="psA")
                    for c in range(0, KE, KSTEP):
                        nc.tensor.matmul(
                            ps[...],
                            kvT[:, c:c + KSTEP, rc * P:(rc + 1) * P],
                            w_sb[f"w{s}"][:, c:c + KSTEP, :],
                            start=(c == 0),
                            stop=(c == KE - KSTEP and not use_c[s]),
                            perf_mode=DR)
                    if use_c[s]:
                        nc.tensor.matmul(ps[...], ones_sb[:, 0:P],
                                         w_sb[f"c{s}"][...],
                                         start=False, stop=True)
                    hp = hpre.tile([P, E], BF16, tag="hpre")
                    nc.vector.tensor_add(hp[...], ps[...], x[:, rc, :])
                    hps.append(hp)
                ln_group(hps, [z[:, rc, :] for rc in range(RC)],
                         eps_sb_a, sc_a)
                if aff_a[s]:
                    h1 = zpool.tile([P, RC, E], BF16, tag="h1")
                    for rc in range(RC):
                        nc.vector.tensor_mul(h1[:, rc, :], z[:, rc, :],
                                             rep[f"ga{s}"][...])
                        nc.vector.tensor_add(h1[:, rc, :], h1[:, rc, :],
                                             rep[f"ba{s}"][...])
                    return z, h1
                return z, z

            def zt_ffn1(s, z):
                """zT via PE transpose, then FFN1 + relu -> gT."""
                zT = ztp.tile([P, KE, R], XDT, tag="zT")
                for c in range(KE):
                    pt = psT.tile([P, R], BF16, tag="psT")
                    for rc in range(RC):
                        nc.tensor.transpose(
                            pt[:, rc * P:(rc + 1) * P],
                            z[:, rc, c * P:(c + 1) * P],
                            ident[...])
                    # fp8: z is at scale 128; zT written at scale 2
                    nc.scalar.activation(out=zT[:, c, :], in_=pt[...],
                                         func=Copy,
                                         scale=(1.0 / 64.0 if fp8 else 1.0))
                gt = gpool.tile([P, KH, R], XDT, tag="gt")
                for j in range(KH):
                    pg = psG.tile([P, R], F32, tag="psG")
                    for c in range(0, KE, KSTEP):
                        nc.tensor.matmul(
                            pg[...],
                            w_sb[f"u{s}"][:, c:c + KSTEP,
                                          j * P:(j + 1) * P],
                            zT[:, c:c + KSTEP, :],
                            start=(c == 0),
                            stop=(c == KE - KSTEP and not use_d[s]),
                            perf_mode=DR)
                    if use_d[s]:
                        nc.tensor.matmul(
                            pg[...], w_sb[f"d{s}"][:, j * P:(j + 1) * P],
                            ones_sb[...], start=False, stop=True)
                    # fp8: psum at scale 128 -> gt at scale 2
                    nc.scalar.activation(out=gt[:, j, :], in_=pg[...],
                                         func=Relu,
                                         scale=(1.0 / 64.0 if fp8 else 1.0))
                return gt

            def ffn2_ln2(s, mt, gt, h1, fine=False):
                """FFN2 + LN2 -> out DMA for stream s.

                The residual add runs on the PE (pf += I.T @ h1) so y stays
                fp32 in PSUM; LN2 is per-rc so each PSUM bank frees right
                after its normalize.  fine=True (last tile): per-rc out DMA
                to shorten the drain tail."""
                o = opool.tile([P, RC, E], BF16, tag="ob")
                for rc in range(RC):
                    pf = psA.tile([P, E], F32, tag="psA")
                    for j in range(0, KH, KSTEP):
                        nc.tensor.matmul(
                            pf[...],
                            gt[:, j:j + KSTEP, rc * P:(rc + 1) * P],
                            w_sb[f"v{s}"][:, j:j + KSTEP, :],
                            start=(j == 0), stop=False,
                            perf_mode=DR)
                    if use_e[s]:
                        nc.tensor.matmul(pf[...], ones_sb[:, 0:P],
                                         w_sb[f"e{s}"][...],
                                         start=False, stop=False)
                    # residual: y = pf + h1 via identity matmul (exact)
                    nc.tensor.matmul(pf[...], ident[...], h1[:, rc, :],
                                     start=False, stop=True)
                    # per-rc LN on the PSUM tile
                    st6 = stats.tile([P, 6], F32, tag="st6")
                    nc.vector.bn_stats(out=st6[...], in_=pf[...])
                    mv = stats.tile([P, 2], F32, tag="mv")
                    nc.vector.bn_aggr(out=mv[...], in_=st6[...])
                    inv = stats.tile([P, 1], F32, tag="inv")
                    nc.scalar.activation(out=inv[...], in_=mv[:, 1:2],
                                         func=Sqrt, bias=eps_sb_b[...],
                                         scale=sc_b)
                    nc.vector.reciprocal(out=inv[...], in_=inv[...])
                    if aff_b[s]:
                        of = opool.tile([P, E], F32, tag="of")
                        nc.vector.tensor_scalar(
                            out=of[...], in0=pf[...],
                            scalar1=mv[:, 0:1], scalar2=inv[...],
                            op0=SUB, op1=MULT)
                        nc.vector.tensor_mul(of[...], of[...],
                                             rep[f"gb{s}"][...])
                        nc.vector.tensor_add(o[:, rc, :], of[...],
                                             rep[f"bb{s}"][...])
                    else:
                        nc.vector.tensor_scalar(
                            out=o[:, rc, :], in0=pf[...],
                            scalar1=mv[:, 0:1], scalar2=inv[...],
                            op0=SUB, op1=MULT)
                    if fine:
                        nc.scalar.dma_start(out=out[mt, s, :, rc, :],
                                            in_=o[:, rc, :])
                if not fine:
                    h = RC // 2 if RC >= 2 else 1
                    nc.scalar.dma_start(out=out[mt, s, :, :h, :],
                                        in_=o[:, :h, :])
                    if RC >= 2:
                        nc.scalar.dma_start(out=out[mt, s, :, h:, :],
                                            in_=o[:, h:, :])

            def pe_warm(n):
                """dummy N=128 matmuls: keep the PE p-state/clock ramped
                while startup DMAs starve the PE of real work."""
                wm = psA.tile([P, E], F32, tag="psA")
                for _ in range(n):
                    nc.tensor.matmul(wm[:, 0:P], ident[...], ident[...],
                                     start=True, stop=True)

            # --- main loop, FFN2 software-pipelined one tile deep ---
            pe_warm(96)
            pend = None
            for mt in range(NT):
                warm = 16 if mt < 1 else 0
                z0, h10 = attn_ln1(0, x_s, xT_s)
                if warm:
                    pe_warm(warm)
                z1, h11 = attn_ln1(1, x_s, xT_s)
                if warm:
                    pe_warm(warm)
                if pend is not None:
                    pmt, pg0, ph0, pg1, ph1 = pend
                    ffn2_ln2(0, pmt, pg0, ph0)
                    ffn2_ln2(1, pmt, pg1, ph1)
                if mt + 1 < NT:
                    x_s, xT_s = loads(mt + 1)
                gt0 = zt_ffn1(0, z0)
                if warm:
                    pe_warm(warm)
                gt1 = zt_ffn1(1, z1)
                if warm:
                    pe_warm(warm)
                pend = (mt, gt0, h10, gt1, h11)
            pmt, pg0, ph0, pg1, ph1 = pend
            ffn2_ln2(0, pmt, pg0, ph0, fine=True)
            ffn2_ln2(1, pmt, pg1, ph1, fine=True)

    nc.compile()
    return nc


def _prep_host(inputs, fp8):
    """Fold weights host-side; returns (inputs, weight arrays, flags)."""
    g = {k: np.asarray(v, dtype=np.float32) for k, v in inputs.items()}
    wdt = F8 if fp8 else BF
    wscale = 64.0 if fp8 else 1.0      # weight pre-scale (fp8 range use)
    bscale_c = 64.0 if fp8 else 1.0    # attn psum scale
    bscale_de = 128.0 if fp8 else 1.0  # ffn psum / residual scale

    def trivial(a, val):
        return bool(np.all(a == val))

    def kchunks(a, nk, dt):
        # [K, N] -> [P, nk, N] (chunk c = rows c*P:(c+1)*P)
        k, n = a.shape
        assert k == nk * P
        return np.ascontiguousarray(
            a.reshape(nk, P, n).transpose(1, 0, 2) * wscale).astype(dt)

    arrs = {}
    flags = []
    for s, (aw, ab, ow, ob, lna_g, lna_b, lnb_g, lnb_b, w1, b1, w2, b2) in \
            enumerate((
                (g["a1_in_w"], g["a1_in_b"], g["a1_out_w"], g["a1_out_b"],
                 g["ln1_g"], g["ln1_b"], g["ln3_g"], g["ln3_b"],
                 g["f1_w1"], g["f1_b1"], g["f1_w2"], g["f1_b2"]),
                (g["a2_in_w"], g["a2_in_b"], g["a2_out_w"], g["a2_out_b"],
                 g["ln2_g"], g["ln2_b"], g["ln4_g"], g["ln4_b"],
                 g["f2_w1"], g["f2_b1"], g["f2_w2"], g["f2_b2"]))):
        wv = aw[2 * E:3 * E]
        bv = ab[2 * E:3 * E]
        W = ow @ wv                      # [E, E]; attn = kv @ W.T + c
        c = ow @ bv + ob                 # [E]
        U = w1 * lna_g[None, :]          # LN1 gain folded into FFN1
        d = b1 + w1 @ lna_b              # LN1 bias folded into FFN1 bias
        V = w2                           # [E, HID]
        e = b2                           # [E]
        arrs[f"w{s}"] = kchunks(W.T, E // P, wdt)
        arrs[f"u{s}"] = kchunks(U.T, E // P, wdt)
        arrs[f"v{s}"] = kchunks(V.T, HID // P, wdt)
        uc = not trivial(c, 0.0)
        ud = not trivial(d, 0.0)
        ue = not trivial(e, 0.0)
        fa = not (trivial(lna_g, 1.0) and trivial(lna_b, 0.0))
        fb = not (trivial(lnb_g, 1.0) and trivial(lnb_b, 0.0))
        if uc:
            arrs[f"c{s}"] = (bscale_c * c).reshape(1, E).astype(BF)
        if ud:
            arrs[f"d{s}"] = (bscale_de * d).reshape(1, HID).astype(BF)
        if ue:
            arrs[f"e{s}"] = (bscale_de * e).reshape(1, E).astype(BF)
        if fa:
            arrs[f"ga{s}"] = lna_g.reshape(1, E).astype(BF)
            arrs[f"ba{s}"] = (bscale_de * lna_b).reshape(1, E).astype(BF)
        if fb:
            arrs[f"gb{s}"] = lnb_g.reshape(1, E).astype(np.float32)
            arrs[f"bb{s}"] = lnb_b.reshape(1, E).astype(np.float32)
        flags.append((uc, ud, ue, fa, fb))

    (uc0, ud0, ue0, fa0, fb0), (uc1, ud1, ue1, fa1, fb1) = flags
    flag_t = (uc0, uc1, ud0, ud1, ue0, ue1, fa0, fa1, fb0, fb1)
    return g, arrs, flag_t


def _pick_rmacro(rows_per_core):
    for r in (512, 256, 128):
        if rows_per_core % r == 0:
            return r
    raise ValueError(f"rows_per_core {rows_per_core} not divisible by 128")


def _core_layouts(x_str, x_tr, sl, NT, R):
    """Per-core straight + transposed DRAM layouts."""
    RC = R // P
    KE = E // P
    xs = np.ascontiguousarray(
        x_str[sl].reshape(NT, RC, P, E).transpose(0, 2, 1, 3))
    xt = np.ascontiguousarray(
        x_tr[sl].reshape(NT, R, KE, P).transpose(0, 3, 2, 1))
    return xs, xt


def prepare(inputs, fp8=USE_FP8):
    """Compile (cached) + build per-core input maps.

    Returns (nc, in_maps, rows_per_core)."""
    g, arrs, flag_t = _prep_host(inputs, fp8)
    B = g["dna"].shape[0]
    rows_per_core = B // NCORES
    rmacro = _pick_rmacro(rows_per_core)
    key = ("v5", fp8, rows_per_core, rmacro, flag_t)
    if key not in _prog_cache:
        _prog_cache[key] = _build_program(rows_per_core, rmacro, flag_t,
                                          fp8)
    nc = _prog_cache[key]

    NT = rows_per_core // rmacro
    xstr = {}
    xtr = {}
    for nm in ("dna", "mol"):
        x = g[nm]
        if fp8:
            xstr[nm] = (64.0 * x).astype(BF)
            xtr[nm] = x.astype(F8)
        else:
            xb = x.astype(BF)
            xstr[nm] = xb
            xtr[nm] = xb
    in_maps = []
    for i in range(NCORES):
        sl = slice(i * rows_per_core, (i + 1) * rows_per_core)
        xs0, xt0 = _core_layouts(xstr["dna"], xtr["dna"], sl, NT, rmacro)
        xs1, xt1 = _core_layouts(xstr["mol"], xtr["mol"], sl, NT, rmacro)
        im = {"xs_dna": xs0, "xt_dna": xt0, "xs_mol": xs1, "xt_mol": xt1}
        im.update(arrs)
        in_maps.append(im)
    return nc, in_maps, rows_per_core


def gather_out(res, rows_per_core):
    outs = []
    for r in res.results:
        o = r["out"]  # [NT, 2, P, RC, E]
        o = o.transpose(0, 3, 2, 1, 4).reshape(rows_per_core, 2 * E)
        outs.append(o)
    return np.concatenate(outs, axis=0).astype(np.float32)


def kernel(**inputs):
    nc, in_maps, rows_per_core = prepare(inputs)
    res = run_bass_kernel_spmd(nc, in_maps, list(range(NCORES)))
    return gather_out(res, rows_per_core)


# revision 25
# speedup vs baseline: 1.2164x; 1.2164x over previous
"""Trainium2 Bass kernel for nn_CrossAttention (seq_len==1 cross attention,
dual-stream transformer block pair).

Math notes (exact simplifications, valid for any input values):
  - Both attentions have seq_len==1 for q and kv, so softmax over the single
    kv position is exactly 1.0 and attention output == V projection:
        mha(q_in, kv_in) = (kv_in @ wv.T + bv) @ out_w.T + out_b
    The q/k projections are dead code.  Folding the two matmuls:
        attn = kv_in @ (out_w @ wv).T + (out_w @ bv + out_b)
  - LayerNorm affine (g, b) of ln1/ln2 is folded into the following FFN
    weights host-side; residual-path affine and biases are applied on-device
    only when they are non-trivial (they are zeros/ones for the reference
    setup_inputs, so the fast path emits no extra instructions).
  - LayerNorm is scale invariant: LN(c*x) == LN(x).  This lets per-tensor
    pow2 scales (used by the fp8 path) cancel without any rescale ops.

v4 layout: host pre-transposes dna/mol (and pre-permutes all tile layouts)
so every SBUF tile is filled by ONE straight DMA — no xbar DMA transposes.
Identity/constant tiles are created BEFORE any DMA issue so the PE warmup
stream starts immediately instead of queueing behind the weight preload.
Queues: sync = transposed x, gpsimd = straight x, scalar = weights + out.
Per-tile PE stream is software-pipelined one tile deep (FFN2 of tile t-1
runs between attn and zT/FFN1 of tile t).  LN sqrt/recip batched per group.
"""

import numpy as np
import ml_dtypes
from contextlib import ExitStack

import concourse.bass as bass
import concourse.tile as tile
from concourse import bacc, mybir
from concourse.bass_utils import run_bass_kernel_spmd

E = 512
HID = 1024
NCORES = 8
EPS = 1e-5
P = 128

BF16 = mybir.dt.bfloat16
F32 = mybir.dt.float32
FP8 = mybir.dt.float8e4
BF = ml_dtypes.bfloat16
F8 = ml_dtypes.float8_e4m3

# fp8 path: pow2 per-tensor scales, all cancelled by LN scale invariance.
#   straight x: bf16(64*x)          transposed x: fp8(x)
#   W,U,V: fp8(64*W)                => attn psum scale 64, h scale 64
#   z written at scale 128 (inv folded); zT = fp8(z/64) at scale 2
#   gt = fp8(relu(psum1)/64) at scale 2; ffn2 psum scale 2*64 = 128 == z
USE_FP8 = True

_prog_cache = {}


def _build_program(rows_per_core: int, rmacro: int, flags: tuple,
                   fp8: bool):
    """Build + compile the per-core Bass program.

    flags = (use_c0, use_c1, use_d0, use_d1, use_e0, use_e1,
             aff_a0, aff_a1, aff_b0, aff_b1)
    """
    (use_c0, use_c1, use_d0, use_d1, use_e0, use_e1,
     aff_a0, aff_a1, aff_b0, aff_b1) = flags
    use_c = (use_c0, use_c1)
    use_d = (use_d0, use_d1)
    use_e = (use_e0, use_e1)
    aff_a = (aff_a0, aff_a1)
    aff_b = (aff_b0, aff_b1)

    R = rmacro
    NT = rows_per_core // R
    RC = R // P
    KE = E // P    # 4 K-chunks over E
    KH = HID // P  # 8 K-chunks over HID
    XDT = FP8 if fp8 else BF16
    DR = mybir.MatmulPerfMode.DoubleRow if fp8 else None
    KSTEP = 2 if fp8 else 1

    nc = bacc.Bacc("TRN2", target_bir_lowering=False, debug=False,
                   num_devices=NCORES)

    # straight x: xs[mt, p, rc, e] = x[mt*R + rc*P + p, e]
    # transposed x: xt[mt, p, c, r] = x[mt*R + r, c*P + p]
    xs_d = {}
    xt_d = {}
    for s, nm in ((0, "dna"), (1, "mol")):
        xs_d[s] = nc.dram_tensor(f"xs_{nm}", [NT, P, RC, E], BF16,
                                 kind="ExternalInput").ap()
        xt_d[s] = nc.dram_tensor(f"xt_{nm}", [NT, P, KE, R], XDT,
                                 kind="ExternalInput").ap()
    # out[mt, s, p, rc, e] = result[mt*R + rc*P + p, s*E + e]
    out = nc.dram_tensor("out", [NT, 2, P, RC, E], BF16,
                         kind="ExternalOutput").ap()

    wts = {}
    for s in range(2):
        wts[f"w{s}"] = nc.dram_tensor(f"w{s}", [P, KE, E], XDT,
                                      kind="ExternalInput").ap()
        wts[f"u{s}"] = nc.dram_tensor(f"u{s}", [P, KE, HID], XDT,
                                      kind="ExternalInput").ap()
        wts[f"v{s}"] = nc.dram_tensor(f"v{s}", [P, KH, E], XDT,
                                      kind="ExternalInput").ap()
        if use_c[s]:
            wts[f"c{s}"] = nc.dram_tensor(f"c{s}", [1, E], BF16,
                                          kind="ExternalInput").ap()
        if use_d[s]:
            wts[f"d{s}"] = nc.dram_tensor(f"d{s}", [1, HID], BF16,
                                          kind="ExternalInput").ap()
        if use_e[s]:
            wts[f"e{s}"] = nc.dram_tensor(f"e{s}", [1, E], BF16,
                                          kind="ExternalInput").ap()
        if aff_a[s]:
            wts[f"ga{s}"] = nc.dram_tensor(f"ga{s}", [1, E], BF16,
                                           kind="ExternalInput").ap()
            wts[f"ba{s}"] = nc.dram_tensor(f"ba{s}", [1, E], BF16,
                                           kind="ExternalInput").ap()
        if aff_b[s]:
            wts[f"gb{s}"] = nc.dram_tensor(f"gb{s}", [1, E], F32,
                                           kind="ExternalInput").ap()
            wts[f"bb{s}"] = nc.dram_tensor(f"bb{s}", [1, E], F32,
                                           kind="ExternalInput").ap()

    SUB = mybir.AluOpType.subtract
    MULT = mybir.AluOpType.mult
    Relu = mybir.ActivationFunctionType.Relu
    Sqrt = mybir.ActivationFunctionType.Sqrt
    Copy = mybir.ActivationFunctionType.Copy
    Ident = mybir.ActivationFunctionType.Identity

    with tile.TileContext(nc) as tc:
        with ExitStack() as ctx:
            const = ctx.enter_context(tc.tile_pool(name="const", bufs=1))
            xin = ctx.enter_context(tc.tile_pool(name="xin", bufs=4))
            xtp = ctx.enter_context(tc.tile_pool(name="xtp", bufs=4))
            hpre = ctx.enter_context(tc.tile_pool(name="hpre", bufs=10))
            zpool = ctx.enter_context(tc.tile_pool(name="zpool", bufs=6))
            ztp = ctx.enter_context(tc.tile_pool(name="ztp", bufs=4))
            gpool = ctx.enter_context(tc.tile_pool(name="gpool", bufs=4))
            ypool = ctx.enter_context(tc.tile_pool(name="ypool", bufs=10))
            opool = ctx.enter_context(tc.tile_pool(name="opool", bufs=4))
            stats = ctx.enter_context(tc.tile_pool(name="stats", bufs=40))
            psA = ctx.enter_context(
                tc.tile_pool(name="psA", bufs=4, space="PSUM"))
            psG = ctx.enter_context(
                tc.tile_pool(name="psG", bufs=2, space="PSUM"))
            psT = ctx.enter_context(
                tc.tile_pool(name="psT", bufs=2, space="PSUM"))

            # --- consts FIRST: PE warmup must not wait on the DMA preload ---
            ident = const.tile([P, P], BF16, tag="ident")
            from concourse.masks import make_identity
            make_identity(nc, ident[...])
            # LN sqrt args (fp8: h at scale 64, y at scale 128):
            #   LN1: 1/inv = sqrt(var_h/16384 + eps/4)   -> z at scale 128
            #   LN2: 1/inv = sqrt(var_y + 16384*eps)     -> out at scale 1
            if fp8:
                eps_a, sc_a = EPS / 4.0, 1.0 / 16384.0
                eps_b, sc_b = EPS * 16384.0, 1.0
            else:
                eps_a, sc_a = EPS, 1.0
                eps_b, sc_b = EPS, 1.0
            eps_sb_a = const.tile([P, 1], F32, tag="eps_a")
            nc.vector.memset(eps_sb_a[...], eps_a)
            eps_sb_b = const.tile([P, 1], F32, tag="eps_b")
            nc.vector.memset(eps_sb_b[...], eps_b)
            ones_sb = const.tile([1, R], BF16, tag="ones")
            nc.vector.memset(ones_sb[...], 1.0)

            w_sb = {}
            for name, ap in wts.items():
                t = const.tile(list(ap.shape), ap.dtype, tag=f"w_{name}")
                w_sb[name] = t

            def loads(mt):
                """issue straight + transposed loads of both streams.

                sync: transposed x (mol first: attn s0 needs molT).
                gpsimd: straight x (dna first: residual s0 consumes it first).
                """
                xt_m = xtp.tile([P, KE, R], XDT, tag="xT")
                xt_dd = xtp.tile([P, KE, R], XDT, tag="xT")
                nc.sync.dma_start(out=xt_m[...], in_=xt_d[1][mt])
                nc.sync.dma_start(out=xt_dd[...], in_=xt_d[0][mt])
                x_dd = xin.tile([P, RC, E], BF16, tag="xin")
                x_m = xin.tile([P, RC, E], BF16, tag="xin")
                nc.gpsimd.dma_start(out=x_dd[...], in_=xs_d[0][mt])
                nc.gpsimd.dma_start(out=x_m[...], in_=xs_d[1][mt])
                return [x_dd, x_m], [xt_dd, xt_m]

            # --- startup issue schedule (per-queue ~85 GB/s, ~12us/MB) ---
            # PE needs (us, approx): w0@12 w1@15 u0@19 u1@26 v0@48 v1@55.
            nc.scalar.dma_start(out=w_sb["w0"][...], in_=wts["w0"])
            x_s, xT_s = loads(0)
            nc.scalar.dma_start(out=w_sb["w1"][...], in_=wts["w1"])
            nc.scalar.dma_start(out=w_sb["u0"][:, :2, :],
                                in_=wts["u0"][:, :2, :])
            nc.sync.dma_start(out=w_sb["u0"][:, 2:, :],
                              in_=wts["u0"][:, 2:, :])
            nc.gpsimd.dma_start(out=w_sb["u1"][:, 2:, :],
                                in_=wts["u1"][:, 2:, :])
            nc.sync.dma_start(out=w_sb["u1"][:, :2, :],
                              in_=wts["u1"][:, :2, :])
            nc.scalar.dma_start(out=w_sb["v0"][...], in_=wts["v0"])
            nc.gpsimd.dma_start(out=w_sb["v1"][...], in_=wts["v1"])
            for name in wts:
                if name[0] not in "wuv":
                    nc.scalar.dma_start(out=w_sb[name][...], in_=wts[name])

            # replicated affine tiles (only when needed)
            rep = {}
            for s in range(2):
                if aff_a[s]:
                    for nm in (f"ga{s}", f"ba{s}"):
                        r = const.tile([P, E], BF16, tag=f"rep_{nm}")
                        nc.sync.dma_start(out=r[...],
                                          in_=wts[nm].to_broadcast((P, E)))
                        rep[nm] = r
                if aff_b[s]:
                    for nm in (f"gb{s}", f"bb{s}"):
                        r = const.tile([P, E], F32, tag=f"rep_{nm}")
                        nc.sync.dma_start(out=r[...],
                                          in_=wts[nm].to_broadcast((P, E)))
                        rep[nm] = r

            def ln_group(srcs, dsts, eps_sb, sq_scale):
                """batched LN over RC chunks: one sqrt+recip for the group."""
                mv = stats.tile([P, RC, 2], F32, tag="mv")
                for rc in range(RC):
                    st6 = stats.tile([P, 6], F32, tag="st6")
                    nc.vector.bn_stats(out=st6[...], in_=srcs[rc])
                    nc.vector.bn_aggr(out=mv[:, rc, :], in_=st6[...])
                inv = stats.tile([P, RC, 1], F32, tag="inv")
                nc.scalar.activation(out=inv[...], in_=mv[:, :, 1:2],
                                     func=Sqrt, bias=eps_sb[...],
                                     scale=sq_scale)
                nc.vector.reciprocal(out=inv[...], in_=inv[...])
                for rc in range(RC):
                    nc.vector.tensor_scalar(
                        out=dsts[rc], in0=srcs[rc],
                        scalar1=mv[:, rc, 0:1], scalar2=inv[:, rc, 0:1],
                        op0=SUB, op1=MULT)

            def attn_ln1(s, x_s, xT_s):
                """attn + LN1 -> z (bf16), h1 (residual input for LN2)."""
                x = x_s[s]
                kvT = xT_s[1 - s]
                z = zpool.tile([P, RC, E], BF16, tag="z")
                hps = []
                for rc in range(RC):
                    ps = psA.tile([P, E], F32, tag="psA")
                    for c in range(0, KE, KSTEP):
                        nc.tensor.matmul(
                            ps[...],
                            kvT[:, c:c + KSTEP, rc * P:(rc + 1) * P],
                            w_sb[f"w{s}"][:, c:c + KSTEP, :],
                            start=(c == 0),
                            stop=(c == KE - KSTEP and not use_c[s]),
                            perf_mode=DR)
                    if use_c[s]:
                        nc.tensor.matmul(ps[...], ones_sb[:, 0:P],
                                         w_sb[f"c{s}"][...],
                                         start=False, stop=True)
                    hp = hpre.tile([P, E], BF16, tag="hpre")
                    nc.vector.tensor_add(hp[...], ps[...], x[:, rc, :])
                    hps.append(hp)
                ln_group(hps, [z[:, rc, :] for rc in range(RC)],
                         eps_sb_a, sc_a)
                if aff_a[s]:
                    h1 = zpool.tile([P, RC, E], BF16, tag="h1")
                    for rc in range(RC):
                        nc.vector.tensor_mul(h1[:, rc, :], z[:, rc, :],
                                             rep[f"ga{s}"][...])
                        nc.vector.tensor_add(h1[:, rc, :], h1[:, rc, :],
                                             rep[f"ba{s}"][...])
                    return z, h1
                return z, z

            def zt_ffn1(s, z):
                """zT via PE transpose, then FFN1 + relu -> gT."""
                zT = ztp.tile([P, KE, R], XDT, tag="zT")
                for c in range(KE):
                    pt = psT.tile([P, R], BF16, tag="psT")
                    for rc in range(RC):
                        nc.tensor.transpose(
                            pt[:, rc * P:(rc + 1) * P],
                            z[:, rc, c * P:(c + 1) * P],
                            ident[...])
                    # fp8: z is at scale 128; zT written at scale 2
                    nc.scalar.activation(out=zT[:, c, :], in_=pt[...],
                                         func=Copy,
                                         scale=(1.0 / 64.0 if fp8 else 1.0))
                gt = gpool.tile([P, KH, R], XDT, tag="gt")
                for j in range(KH):
                    pg = psG.tile([P, R], F32, tag="psG")
                    for c in range(0, KE, KSTEP):
                        nc.tensor.matmul(
                            pg[...],
                            w_sb[f"u{s}"][:, c:c + KSTEP,
                                          j * P:(j + 1) * P],
                            zT[:, c:c + KSTEP, :],
                            start=(c == 0),
                            stop=(c == KE - KSTEP and not use_d[s]),
                            perf_mode=DR)
                    if use_d[s]:
                        nc.tensor.matmul(
                            pg[...], w_sb[f"d{s}"][:, j * P:(j + 1) * P],
                            ones_sb[...], start=False, stop=True)
                    # fp8: psum at scale 128 -> gt at scale 2
                    nc.scalar.activation(out=gt[:, j, :], in_=pg[...],
                                         func=Relu,
                                         scale=(1.0 / 64.0 if fp8 else 1.0))
                return gt

            def ffn2_ln2(s, mt, gt, h1, fine=False):
                """FFN2 + LN2 -> out DMA for stream s.

                y-add on DVE (the one PSUM-reading op); LN2 stats on DVE
                from bf16 y; the normalize itself runs on ACT (Lrelu
                alpha=1 as identity, scale=1/std, bias=-mean/std) to
                offload DVE.  fine=True (last tile): per-rc out DMA."""
                ys = []
                for rc in range(RC):
                    pf = psA.tile([P, E], F32, tag="psA")
                    for j in range(0, KH, KSTEP):
                        nc.tensor.matmul(
                            pf[...],
                            gt[:, j:j + KSTEP, rc * P:(rc + 1) * P],
                            w_sb[f"v{s}"][:, j:j + KSTEP, :],
                            start=(j == 0),
                            stop=(j == KH - KSTEP and not use_e[s]),
                            perf_mode=DR)
                    if use_e[s]:
                        nc.tensor.matmul(pf[...], ones_sb[:, 0:P],
                                         w_sb[f"e{s}"][...],
                                         start=False, stop=True)
                    y = ypool.tile([P, E], BF16, tag="y")
                    nc.vector.tensor_add(y[...], pf[...], h1[:, rc, :])
                    ys.append(y)
                # group stats; normalize on ACT with scale/bias per rc
                mv = stats.tile([P, RC, 2], F32, tag="mv")
                for rc in range(RC):
                    st6 = stats.tile([P, 6], F32, tag="st6")
                    nc.vector.bn_stats(out=st6[...], in_=ys[rc][...])
                    nc.vector.bn_aggr(out=mv[:, rc, :], in_=st6[...])
                inv = stats.tile([P, RC, 1], F32, tag="inv")
                nc.scalar.activation(out=inv[...], in_=mv[:, :, 1:2],
                                     func=Sqrt, bias=eps_sb_b[...],
                                     scale=sc_b)
                nc.vector.reciprocal(out=inv[...], in_=inv[...])
                nb = stats.tile([P, RC, 1], F32, tag="nb")
                nc.vector.tensor_tensor(out=nb[...], in0=mv[:, :, 0:1],
                                        in1=inv[...], op=MULT)
                nc.vector.tensor_scalar_mul(out=nb[...], in0=nb[...],
                                            scalar1=-1.0)
                o = opool.tile([P, RC, E], BF16, tag="ob")
                for rc in range(RC):
                    dst = o[:, rc, :]
                    if aff_b[s]:
                        of = opool.tile([P, E], F32, tag="of")
                        nc.scalar.activation(
                            out=of[...], in_=ys[rc][...], func=Ident,
                            bias=nb[:, rc, 0:1], scale=inv[:, rc, 0:1])
                        nc.vector.tensor_mul(of[...], of[...],
                                             rep[f"gb{s}"][...])
                        nc.vector.tensor_add(dst, of[...],
                                             rep[f"bb{s}"][...])
                    else:
                        nc.scalar.activation(
                            out=dst, in_=ys[rc][...], func=Ident,
                            bias=nb[:, rc, 0:1], scale=inv[:, rc, 0:1])
                    if fine:
                        nc.scalar.dma_start(out=out[mt, s, :, rc, :],
                                            in_=o[:, rc, :])
                if not fine:
                    h = RC // 2 if RC >= 2 else 1
                    nc.scalar.dma_start(out=out[mt, s, :, :h, :],
                                        in_=o[:, :h, :])
                    if RC >= 2:
                        nc.scalar.dma_start(out=out[mt, s, :, h:, :],
                                            in_=o[:, h:, :])

            def pe_warm(n):
                """dummy N=128 matmuls: keep the PE p-state/clock ramped
                while startup DMAs starve the PE of real work."""
                wm = psA.tile([P, E], F32, tag="psA")
                for _ in range(n):
                    nc.tensor.matmul(wm[:, 0:P], ident[...], ident[...],
                                     start=True, stop=True)

            # --- main loop, FFN2 software-pipelined one tile deep ---
            pe_warm(96)
            pend = None
            for mt in range(NT):
                warm = 16 if mt < 1 else 0
                z0, h10 = attn_ln1(0, x_s, xT_s)
                if warm:
                    pe_warm(warm)
                z1, h11 = attn_ln1(1, x_s, xT_s)
                if warm:
                    pe_warm(warm)
                if pend is not None:
                    pmt, pg0, ph0, pg1, ph1 = pend
                    ffn2_ln2(0, pmt, pg0, ph0)
                    ffn2_ln2(1, pmt, pg1, ph1)
                if mt + 1 < NT:
                    x_s, xT_s = loads(mt + 1)
                gt0 = zt_ffn1(0, z0)
                if warm:
                    pe_warm(warm)
                gt1 = zt_ffn1(1, z1)
                if warm:
                    pe_warm(warm)
                pend = (mt, gt0, h10, gt1, h11)
            pmt, pg0, ph0, pg1, ph1 = pend
            ffn2_ln2(0, pmt, pg0, ph0, fine=True)
            ffn2_ln2(1, pmt, pg1, ph1, fine=True)

    nc.compile()
    return nc


def _prep_host(inputs, fp8):
    """Fold weights host-side; returns (inputs, weight arrays, flags)."""
    g = {k: np.asarray(v, dtype=np.float32) for k, v in inputs.items()}
    wdt = F8 if fp8 else BF
    wscale = 64.0 if fp8 else 1.0      # weight pre-scale (fp8 range use)
    bscale_c = 64.0 if fp8 else 1.0    # attn psum scale
    bscale_de = 128.0 if fp8 else 1.0  # ffn psum / residual scale

    def trivial(a, val):
        return bool(np.all(a == val))

    def kchunks(a, nk, dt):
        # [K, N] -> [P, nk, N] (chunk c = rows c*P:(c+1)*P)
        k, n = a.shape
        assert k == nk * P
        return np.ascontiguousarray(
            a.reshape(nk, P, n).transpose(1, 0, 2) * wscale).astype(dt)

    arrs = {}
    flags = []
    for s, (aw, ab, ow, ob, lna_g, lna_b, lnb_g, lnb_b, w1, b1, w2, b2) in \
            enumerate((
                (g["a1_in_w"], g["a1_in_b"], g["a1_out_w"], g["a1_out_b"],
                 g["ln1_g"], g["ln1_b"], g["ln3_g"], g["ln3_b"],
                 g["f1_w1"], g["f1_b1"], g["f1_w2"], g["f1_b2"]),
                (g["a2_in_w"], g["a2_in_b"], g["a2_out_w"], g["a2_out_b"],
                 g["ln2_g"], g["ln2_b"], g["ln4_g"], g["ln4_b"],
                 g["f2_w1"], g["f2_b1"], g["f2_w2"], g["f2_b2"]))):
        wv = aw[2 * E:3 * E]
        bv = ab[2 * E:3 * E]
        W = ow @ wv                      # [E, E]; attn = kv @ W.T + c
        c = ow @ bv + ob                 # [E]
        U = w1 * lna_g[None, :]          # LN1 gain folded into FFN1
        d = b1 + w1 @ lna_b              # LN1 bias folded into FFN1 bias
        V = w2                           # [E, HID]
        e = b2                           # [E]
        arrs[f"w{s}"] = kchunks(W.T, E // P, wdt)
        arrs[f"u{s}"] = kchunks(U.T, E // P, wdt)
        arrs[f"v{s}"] = kchunks(V.T, HID // P, wdt)
        uc = not trivial(c, 0.0)
        ud = not trivial(d, 0.0)
        ue = not trivial(e, 0.0)
        fa = not (trivial(lna_g, 1.0) and trivial(lna_b, 0.0))
        fb = not (trivial(lnb_g, 1.0) and trivial(lnb_b, 0.0))
        if uc:
            arrs[f"c{s}"] = (bscale_c * c).reshape(1, E).astype(BF)
        if ud:
            arrs[f"d{s}"] = (bscale_de * d).reshape(1, HID).astype(BF)
        if ue:
            arrs[f"e{s}"] = (bscale_de * e).reshape(1, E).astype(BF)
        if fa:
            arrs[f"ga{s}"] = lna_g.reshape(1, E).astype(BF)
            arrs[f"ba{s}"] = (bscale_de * lna_b).reshape(1, E).astype(BF)
        if fb:
            arrs[f"gb{s}"] = lnb_g.reshape(1, E).astype(np.float32)
            arrs[f"bb{s}"] = lnb_b.reshape(1, E).astype(np.float32)
        flags.append((uc, ud, ue, fa, fb))

    (uc0, ud0, ue0, fa0, fb0), (uc1, ud1, ue1, fa1, fb1) = flags
    flag_t = (uc0, uc1, ud0, ud1, ue0, ue1, fa0, fa1, fb0, fb1)
    return g, arrs, flag_t


def _pick_rmacro(rows_per_core):
    for r in (512, 256, 128):
        if rows_per_core % r == 0:
            return r
    raise ValueError(f"rows_per_core {rows_per_core} not divisible by 128")


def _core_layouts(x_str, x_tr, sl, NT, R):
    """Per-core straight + transposed DRAM layouts."""
    RC = R // P
    KE = E // P
    xs = np.ascontiguousarray(
        x_str[sl].reshape(NT, RC, P, E).transpose(0, 2, 1, 3))
    xt = np.ascontiguousarray(
        x_tr[sl].reshape(NT, R, KE, P).transpose(0, 3, 2, 1))
    return xs, xt


def prepare(inputs, fp8=USE_FP8):
    """Compile (cached) + build per-core input maps.

    Returns (nc, in_maps, rows_per_core)."""
    g, arrs, flag_t = _prep_host(inputs, fp8)
    B = g["dna"].shape[0]
    rows_per_core = B // NCORES
    rmacro = _pick_rmacro(rows_per_core)
    key = ("v5", fp8, rows_per_core, rmacro, flag_t)
    if key not in _prog_cache:
        _prog_cache[key] = _build_program(rows_per_core, rmacro, flag_t,
                                          fp8)
    nc = _prog_cache[key]

    NT = rows_per_core // rmacro
    xstr = {}
    xtr = {}
    for nm in ("dna", "mol"):
        x = g[nm]
        if fp8:
            xstr[nm] = (64.0 * x).astype(BF)
            xtr[nm] = x.astype(F8)
        else:
            xb = x.astype(BF)
            xstr[nm] = xb
            xtr[nm] = xb
    in_maps = []
    for i in range(NCORES):
        sl = slice(i * rows_per_core, (i + 1) * rows_per_core)
        xs0, xt0 = _core_layouts(xstr["dna"], xtr["dna"], sl, NT, rmacro)
        xs1, xt1 = _core_layouts(xstr["mol"], xtr["mol"], sl, NT, rmacro)
        im = {"xs_dna": xs0, "xt_dna": xt0, "xs_mol": xs1, "xt_mol": xt1}
        im.update(arrs)
        in_maps.append(im)
    return nc, in_maps, rows_per_core


def gather_out(res, rows_per_core):
    outs = []
    for r in res.results:
        o = r["out"]  # [NT, 2, P, RC, E]
        o = o.transpose(0, 3, 2, 1, 4).reshape(rows_per_core, 2 * E)
        outs.append(o)
    return np.concatenate(outs, axis=0).astype(np.float32)


def kernel(**inputs):
    nc, in_maps, rows_per_core = prepare(inputs)
    res = run_bass_kernel_spmd(nc, in_maps, list(range(NCORES)))
    return gather_out(res, rows_per_core)


# revision 27
# speedup vs baseline: 1.2305x; 1.0116x over previous
"""Trainium2 Bass kernel for nn_CrossAttention (seq_len==1 cross attention,
dual-stream transformer block pair).

Math notes (exact simplifications, valid for any input values):
  - Both attentions have seq_len==1 for q and kv, so softmax over the single
    kv position is exactly 1.0 and attention output == V projection:
        mha(q_in, kv_in) = (kv_in @ wv.T + bv) @ out_w.T + out_b
    The q/k projections are dead code.  Folding the two matmuls:
        attn = kv_in @ (out_w @ wv).T + (out_w @ bv + out_b)
  - LayerNorm affine (g, b) of ln1/ln2 is folded into the following FFN
    weights host-side; residual-path affine and biases are applied on-device
    only when they are non-trivial (they are zeros/ones for the reference
    setup_inputs, so the fast path emits no extra instructions).
  - LayerNorm is scale invariant: LN(c*x) == LN(x).  This lets per-tensor
    pow2 scales (used by the fp8 path) cancel without any rescale ops.

v4 layout: host pre-transposes dna/mol (and pre-permutes all tile layouts)
so every SBUF tile is filled by ONE straight DMA — no xbar DMA transposes.
Identity/constant tiles are created BEFORE any DMA issue so the PE warmup
stream starts immediately instead of queueing behind the weight preload.
Queues: sync = transposed x, gpsimd = straight x, scalar = weights + out.
Per-tile PE stream is software-pipelined one tile deep (FFN2 of tile t-1
runs between attn and zT/FFN1 of tile t).  LN sqrt/recip batched per group.
"""

import numpy as np
import ml_dtypes
from contextlib import ExitStack

import concourse.bass as bass
import concourse.tile as tile
from concourse import bacc, mybir
from concourse.bass_utils import run_bass_kernel_spmd

E = 512
HID = 1024
NCORES = 8
EPS = 1e-5
P = 128

BF16 = mybir.dt.bfloat16
F32 = mybir.dt.float32
FP8 = mybir.dt.float8e4
BF = ml_dtypes.bfloat16
F8 = ml_dtypes.float8_e4m3

# fp8 path: pow2 per-tensor scales, all cancelled by LN scale invariance.
#   straight x: bf16(64*x)          transposed x: fp8(x)
#   W,U,V: fp8(64*W)                => attn psum scale 64, h scale 64
#   z written at scale 128 (inv folded); zT = fp8(z/64) at scale 2
#   gt = fp8(relu(psum1)/64) at scale 2; ffn2 psum scale 2*64 = 128 == z
USE_FP8 = True

_prog_cache = {}


def _build_program(rows_per_core: int, rmacro: int, flags: tuple,
                   fp8: bool):
    """Build + compile the per-core Bass program.

    flags = (use_c0, use_c1, use_d0, use_d1, use_e0, use_e1,
             aff_a0, aff_a1, aff_b0, aff_b1)
    """
    (use_c0, use_c1, use_d0, use_d1, use_e0, use_e1,
     aff_a0, aff_a1, aff_b0, aff_b1) = flags
    use_c = (use_c0, use_c1)
    use_d = (use_d0, use_d1)
    use_e = (use_e0, use_e1)
    aff_a = (aff_a0, aff_a1)
    aff_b = (aff_b0, aff_b1)

    R = rmacro
    NT = rows_per_core // R
    RC = R // P
    KE = E // P    # 4 K-chunks over E
    KH = HID // P  # 8 K-chunks over HID
    XDT = FP8 if fp8 else BF16
    DR = mybir.MatmulPerfMode.DoubleRow if fp8 else None
    KSTEP = 2 if fp8 else 1

    nc = bacc.Bacc("TRN2", target_bir_lowering=False, debug=False,
                   num_devices=NCORES)

    # straight x: xs[mt, p, rc, e] = x[mt*R + rc*P + p, e]
    # transposed x: xt[mt, p, c, r] = x[mt*R + r, c*P + p]
    xs_d = {}
    xt_d = {}
    for s, nm in ((0, "dna"), (1, "mol")):
        xs_d[s] = nc.dram_tensor(f"xs_{nm}", [NT, P, RC, E], BF16,
                                 kind="ExternalInput").ap()
        xt_d[s] = nc.dram_tensor(f"xt_{nm}", [NT, P, KE, R], XDT,
                                 kind="ExternalInput").ap()
    # out[mt, s, p, rc, e] = result[mt*R + rc*P + p, s*E + e]
    out = nc.dram_tensor("out", [NT, 2, P, RC, E], BF16,
                         kind="ExternalOutput").ap()

    wts = {}
    for s in range(2):
        wts[f"w{s}"] = nc.dram_tensor(f"w{s}", [P, KE, E], XDT,
                                      kind="ExternalInput").ap()
        wts[f"u{s}"] = nc.dram_tensor(f"u{s}", [P, KE, HID], XDT,
                                      kind="ExternalInput").ap()
        wts[f"v{s}"] = nc.dram_tensor(f"v{s}", [P, KH, E], XDT,
                                      kind="ExternalInput").ap()
        if use_c[s]:
            wts[f"c{s}"] = nc.dram_tensor(f"c{s}", [1, E], BF16,
                                          kind="ExternalInput").ap()
        if use_d[s]:
            wts[f"d{s}"] = nc.dram_tensor(f"d{s}", [1, HID], BF16,
                                          kind="ExternalInput").ap()
        if use_e[s]:
            wts[f"e{s}"] = nc.dram_tensor(f"e{s}", [1, E], BF16,
                                          kind="ExternalInput").ap()
        if aff_a[s]:
            wts[f"ga{s}"] = nc.dram_tensor(f"ga{s}", [1, E], BF16,
                                           kind="ExternalInput").ap()
            wts[f"ba{s}"] = nc.dram_tensor(f"ba{s}", [1, E], BF16,
                                           kind="ExternalInput").ap()
        if aff_b[s]:
            wts[f"gb{s}"] = nc.dram_tensor(f"gb{s}", [1, E], F32,
                                           kind="ExternalInput").ap()
            wts[f"bb{s}"] = nc.dram_tensor(f"bb{s}", [1, E], F32,
                                           kind="ExternalInput").ap()

    SUB = mybir.AluOpType.subtract
    MULT = mybir.AluOpType.mult
    Relu = mybir.ActivationFunctionType.Relu
    Sqrt = mybir.ActivationFunctionType.Sqrt
    Copy = mybir.ActivationFunctionType.Copy
    Ident = mybir.ActivationFunctionType.Identity

    with tile.TileContext(nc) as tc:
        with ExitStack() as ctx:
            const = ctx.enter_context(tc.tile_pool(name="const", bufs=1))
            xin = ctx.enter_context(tc.tile_pool(name="xin", bufs=4))
            xtp = ctx.enter_context(tc.tile_pool(name="xtp", bufs=4))
            hpre = ctx.enter_context(tc.tile_pool(name="hpre", bufs=10))
            zpool = ctx.enter_context(tc.tile_pool(name="zpool", bufs=6))
            ztp = ctx.enter_context(tc.tile_pool(name="ztp", bufs=4))
            gpool = ctx.enter_context(tc.tile_pool(name="gpool", bufs=4))
            ypool = ctx.enter_context(tc.tile_pool(name="ypool", bufs=10))
            opool = ctx.enter_context(tc.tile_pool(name="opool", bufs=4))
            stats = ctx.enter_context(tc.tile_pool(name="stats", bufs=40))
            psA = ctx.enter_context(
                tc.tile_pool(name="psA", bufs=4, space="PSUM"))
            psG = ctx.enter_context(
                tc.tile_pool(name="psG", bufs=2, space="PSUM"))
            psT = ctx.enter_context(
                tc.tile_pool(name="psT", bufs=2, space="PSUM"))

            # --- consts FIRST: PE warmup must not wait on the DMA preload ---
            ident = const.tile([P, P], BF16, tag="ident")
            from concourse.masks import make_identity
            make_identity(nc, ident[...])
            # LN sqrt args (fp8: h at scale 64, y at scale 128):
            #   LN1: 1/inv = sqrt(var_h/16384 + eps/4)   -> z at scale 128
            #   LN2: 1/inv = sqrt(var_y + 16384*eps)     -> out at scale 1
            if fp8:
                eps_a, sc_a = EPS / 4.0, 1.0 / 16384.0
                eps_b, sc_b = EPS * 16384.0, 1.0
            else:
                eps_a, sc_a = EPS, 1.0
                eps_b, sc_b = EPS, 1.0
            eps_sb_a = const.tile([P, 1], F32, tag="eps_a")
            nc.vector.memset(eps_sb_a[...], eps_a)
            eps_sb_b = const.tile([P, 1], F32, tag="eps_b")
            nc.vector.memset(eps_sb_b[...], eps_b)
            ones_sb = const.tile([1, R], BF16, tag="ones")
            nc.vector.memset(ones_sb[...], 1.0)

            w_sb = {}
            for name, ap in wts.items():
                t = const.tile(list(ap.shape), ap.dtype, tag=f"w_{name}")
                w_sb[name] = t

            def loads(mt):
                """issue straight + transposed loads of both streams.

                sync: transposed x (mol first: attn s0 needs molT).
                gpsimd: straight x (dna first: residual s0 consumes it first).
                """
                xt_m = xtp.tile([P, KE, R], XDT, tag="xT")
                xt_dd = xtp.tile([P, KE, R], XDT, tag="xT")
                nc.sync.dma_start(out=xt_m[...], in_=xt_d[1][mt])
                nc.sync.dma_start(out=xt_dd[...], in_=xt_d[0][mt])
                x_dd = xin.tile([P, RC, E], BF16, tag="xin")
                x_m = xin.tile([P, RC, E], BF16, tag="xin")
                nc.gpsimd.dma_start(out=x_dd[...], in_=xs_d[0][mt])
                nc.gpsimd.dma_start(out=x_m[...], in_=xs_d[1][mt])
                return [x_dd, x_m], [xt_dd, xt_m]

            # --- startup issue schedule (per-queue ~85 GB/s, ~12us/MB) ---
            # PE needs (us, approx): w0@12 w1@15 u0@19 u1@26 v0@48 v1@55.
            nc.scalar.dma_start(out=w_sb["w0"][...], in_=wts["w0"])
            x_s, xT_s = loads(0)
            nc.scalar.dma_start(out=w_sb["w1"][...], in_=wts["w1"])
            nc.scalar.dma_start(out=w_sb["u0"][:, :2, :],
                                in_=wts["u0"][:, :2, :])
            nc.sync.dma_start(out=w_sb["u0"][:, 2:, :],
                              in_=wts["u0"][:, 2:, :])
            nc.gpsimd.dma_start(out=w_sb["u1"][:, 2:, :],
                                in_=wts["u1"][:, 2:, :])
            nc.sync.dma_start(out=w_sb["u1"][:, :2, :],
                              in_=wts["u1"][:, :2, :])
            nc.scalar.dma_start(out=w_sb["v0"][...], in_=wts["v0"])
            nc.gpsimd.dma_start(out=w_sb["v1"][...], in_=wts["v1"])
            for name in wts:
                if name[0] not in "wuv":
                    nc.scalar.dma_start(out=w_sb[name][...], in_=wts[name])

            # replicated affine tiles (only when needed)
            rep = {}
            for s in range(2):
                if aff_a[s]:
                    for nm in (f"ga{s}", f"ba{s}"):
                        r = const.tile([P, E], BF16, tag=f"rep_{nm}")
                        nc.sync.dma_start(out=r[...],
                                          in_=wts[nm].to_broadcast((P, E)))
                        rep[nm] = r
                if aff_b[s]:
                    for nm in (f"gb{s}", f"bb{s}"):
                        r = const.tile([P, E], F32, tag=f"rep_{nm}")
                        nc.sync.dma_start(out=r[...],
                                          in_=wts[nm].to_broadcast((P, E)))
                        rep[nm] = r

            def ln_group(srcs, dsts, eps_sb, sq_scale):
                """batched LN over RC chunks: one sqrt+recip for the group."""
                mv = stats.tile([P, RC, 2], F32, tag="mv")
                for rc in range(RC):
                    st6 = stats.tile([P, 6], F32, tag="st6")
                    nc.vector.bn_stats(out=st6[...], in_=srcs[rc])
                    nc.vector.bn_aggr(out=mv[:, rc, :], in_=st6[...])
                inv = stats.tile([P, RC, 1], F32, tag="inv")
                nc.scalar.activation(out=inv[...], in_=mv[:, :, 1:2],
                                     func=Sqrt, bias=eps_sb[...],
                                     scale=sq_scale)
                nc.vector.reciprocal(out=inv[...], in_=inv[...])
                for rc in range(RC):
                    nc.vector.tensor_scalar(
                        out=dsts[rc], in0=srcs[rc],
                        scalar1=mv[:, rc, 0:1], scalar2=inv[:, rc, 0:1],
                        op0=SUB, op1=MULT)

            def attn_ln1(s, x_s, xT_s):
                """attn + LN1 -> z (bf16), h1 (residual input for LN2)."""
                x = x_s[s]
                kvT = xT_s[1 - s]
                z = zpool.tile([P, RC, E], BF16, tag="z")
                hps = []
                for rc in range(RC):
                    ps = psA.tile([P, E], F32, tag="psA")
                    for c in range(0, KE, KSTEP):
                        nc.tensor.matmul(
                            ps[...],
                            kvT[:, c:c + KSTEP, rc * P:(rc + 1) * P],
                            w_sb[f"w{s}"][:, c:c + KSTEP, :],
                            start=(c == 0),
                            stop=(c == KE - KSTEP and not use_c[s]),
                            perf_mode=DR)
                    if use_c[s]:
                        nc.tensor.matmul(ps[...], ones_sb[:, 0:P],
                                         w_sb[f"c{s}"][...],
                                         start=False, stop=True)
                    hp = hpre.tile([P, E], BF16, tag="hpre")
                    nc.vector.tensor_add(hp[...], ps[...], x[:, rc, :])
                    hps.append(hp)
                ln_group(hps, [z[:, rc, :] for rc in range(RC)],
                         eps_sb_a, sc_a)
                if aff_a[s]:
                    h1 = zpool.tile([P, RC, E], BF16, tag="h1")
                    for rc in range(RC):
                        nc.vector.tensor_mul(h1[:, rc, :], z[:, rc, :],
                                             rep[f"ga{s}"][...])
                        nc.vector.tensor_add(h1[:, rc, :], h1[:, rc, :],
                                             rep[f"ba{s}"][...])
                    return z, h1
                return z, z

            def zt_ffn1(s, z):
                """zT via PE transpose, then FFN1 + relu -> gT."""
                zT = ztp.tile([P, KE, R], XDT, tag="zT")
                for c in range(KE):
                    pt = psT.tile([P, R], BF16, tag="psT")
                    for rc in range(RC):
                        nc.tensor.transpose(
                            pt[:, rc * P:(rc + 1) * P],
                            z[:, rc, c * P:(c + 1) * P],
                            ident[...])
                    # fp8: z is at scale 128; zT written at scale 2
                    nc.scalar.activation(out=zT[:, c, :], in_=pt[...],
                                         func=Copy,
                                         scale=(1.0 / 64.0 if fp8 else 1.0))
                gt = gpool.tile([P, KH, R], XDT, tag="gt")
                for j in range(KH):
                    pg = psG.tile([P, R], F32, tag="psG")
                    for c in range(0, KE, KSTEP):
                        nc.tensor.matmul(
                            pg[...],
                            w_sb[f"u{s}"][:, c:c + KSTEP,
                                          j * P:(j + 1) * P],
                            zT[:, c:c + KSTEP, :],
                            start=(c == 0),
                            stop=(c == KE - KSTEP and not use_d[s]),
                            perf_mode=DR)
                    if use_d[s]:
                        nc.tensor.matmul(
                            pg[...], w_sb[f"d{s}"][:, j * P:(j + 1) * P],
                            ones_sb[...], start=False, stop=True)
                    # fp8: psum at scale 128 -> gt at scale 2
                    nc.scalar.activation(out=gt[:, j, :], in_=pg[...],
                                         func=Relu,
                                         scale=(1.0 / 64.0 if fp8 else 1.0))
                return gt

            def ffn2_mm(s, gt, h1):
                """FFN2 matmuls + y-add (DVE, the one PSUM read) -> ys."""
                ys = []
                for rc in range(RC):
                    pf = psA.tile([P, E], F32, tag="psA")
                    for j in range(0, KH, KSTEP):
                        nc.tensor.matmul(
                            pf[...],
                            gt[:, j:j + KSTEP, rc * P:(rc + 1) * P],
                            w_sb[f"v{s}"][:, j:j + KSTEP, :],
                            start=(j == 0),
                            stop=(j == KH - KSTEP and not use_e[s]),
                            perf_mode=DR)
                    if use_e[s]:
                        nc.tensor.matmul(pf[...], ones_sb[:, 0:P],
                                         w_sb[f"e{s}"][...],
                                         start=False, stop=True)
                    y = ypool.tile([P, E], BF16, tag="y")
                    nc.vector.tensor_add(y[...], pf[...], h1[:, rc, :])
                    ys.append(y)
                return ys

            def ln2_fin(s, mt, ys, fine=False):
                """LN2 stats (DVE) + normalize (ACT identity with per-row
                scale/bias) + out DMA (sync queue).  Issued AFTER the next
                tile's zT/FFN1 so ACT's copies/relus aren't blocked."""
                mv = stats.tile([P, RC, 2], F32, tag="mv")
                for rc in range(RC):
                    st6 = stats.tile([P, 6], F32, tag="st6")
                    nc.vector.bn_stats(out=st6[...], in_=ys[rc][...])
                    nc.vector.bn_aggr(out=mv[:, rc, :], in_=st6[...])
                inv = stats.tile([P, RC, 1], F32, tag="inv")
                nc.scalar.activation(out=inv[...], in_=mv[:, :, 1:2],
                                     func=Sqrt, bias=eps_sb_b[...],
                                     scale=sc_b)
                nc.vector.reciprocal(out=inv[...], in_=inv[...])
                nb = stats.tile([P, RC, 1], F32, tag="nb")
                nc.vector.tensor_tensor(out=nb[...], in0=mv[:, :, 0:1],
                                        in1=inv[...], op=MULT)
                nc.vector.tensor_scalar_mul(out=nb[...], in0=nb[...],
                                            scalar1=-1.0)
                o = opool.tile([P, RC, E], BF16, tag="ob")
                for rc in range(RC):
                    dst = o[:, rc, :]
                    if aff_b[s]:
                        of = opool.tile([P, E], F32, tag="of")
                        nc.scalar.activation(
                            out=of[...], in_=ys[rc][...], func=Ident,
                            bias=nb[:, rc, 0:1], scale=inv[:, rc, 0:1])
                        nc.vector.tensor_mul(of[...], of[...],
                                             rep[f"gb{s}"][...])
                        nc.vector.tensor_add(dst, of[...],
                                             rep[f"bb{s}"][...])
                    else:
                        nc.scalar.activation(
                            out=dst, in_=ys[rc][...], func=Ident,
                            bias=nb[:, rc, 0:1], scale=inv[:, rc, 0:1])
                    if fine:
                        nc.sync.dma_start(out=out[mt, s, :, rc, :],
                                          in_=o[:, rc, :])
                if not fine:
                    h = RC // 2 if RC >= 2 else 1
                    nc.sync.dma_start(out=out[mt, s, :, :h, :],
                                      in_=o[:, :h, :])
                    if RC >= 2:
                        nc.sync.dma_start(out=out[mt, s, :, h:, :],
                                          in_=o[:, h:, :])

            def pe_warm(n):
                """dummy N=128 matmuls: keep the PE p-state/clock ramped
                while startup DMAs starve the PE of real work."""
                wm = psA.tile([P, E], F32, tag="psA")
                for _ in range(n):
                    nc.tensor.matmul(wm[:, 0:P], ident[...], ident[...],
                                     start=True, stop=True)

            # --- main loop, FFN2 software-pipelined one tile deep ---
            pe_warm(96)
            pend = None
            for mt in range(NT):
                warm = 16 if mt < 1 else 0
                z0, h10 = attn_ln1(0, x_s, xT_s)
                if warm:
                    pe_warm(warm)
                z1, h11 = attn_ln1(1, x_s, xT_s)
                if warm:
                    pe_warm(warm)
                if pend is not None:
                    pmt, pg0, ph0, pg1, ph1 = pend
                    ys0 = ffn2_mm(0, pg0, ph0)
                    ys1 = ffn2_mm(1, pg1, ph1)
                else:
                    pmt = ys0 = ys1 = None
                if mt + 1 < NT:
                    x_s, xT_s = loads(mt + 1)
                gt0 = zt_ffn1(0, z0)
                if warm:
                    pe_warm(warm)
                if ys0 is not None:
                    ln2_fin(0, pmt, ys0)
                gt1 = zt_ffn1(1, z1)
                if warm:
                    pe_warm(warm)
                if ys1 is not None:
                    ln2_fin(1, pmt, ys1)
                pend = (mt, gt0, h10, gt1, h11)
            pmt, pg0, ph0, pg1, ph1 = pend
            ys0 = ffn2_mm(0, pg0, ph0)
            ys1 = ffn2_mm(1, pg1, ph1)
            ln2_fin(0, pmt, ys0, fine=True)
            ln2_fin(1, pmt, ys1, fine=True)

    nc.compile()
    return nc


def _prep_host(inputs, fp8):
    """Fold weights host-side; returns (inputs, weight arrays, flags)."""
    g = {k: np.asarray(v, dtype=np.float32) for k, v in inputs.items()}
    wdt = F8 if fp8 else BF
    wscale = 64.0 if fp8 else 1.0      # weight pre-scale (fp8 range use)
    bscale_c = 64.0 if fp8 else 1.0    # attn psum scale
    bscale_de = 128.0 if fp8 else 1.0  # ffn psum / residual scale

    def trivial(a, val):
        return bool(np.all(a == val))

    def kchunks(a, nk, dt):
        # [K, N] -> [P, nk, N] (chunk c = rows c*P:(c+1)*P)
        k, n = a.shape
        assert k == nk * P
        return np.ascontiguousarray(
            a.reshape(nk, P, n).transpose(1, 0, 2) * wscale).astype(dt)

    arrs = {}
    flags = []
    for s, (aw, ab, ow, ob, lna_g, lna_b, lnb_g, lnb_b, w1, b1, w2, b2) in \
            enumerate((
                (g["a1_in_w"], g["a1_in_b"], g["a1_out_w"], g["a1_out_b"],
                 g["ln1_g"], g["ln1_b"], g["ln3_g"], g["ln3_b"],
                 g["f1_w1"], g["f1_b1"], g["f1_w2"], g["f1_b2"]),
                (g["a2_in_w"], g["a2_in_b"], g["a2_out_w"], g["a2_out_b"],
                 g["ln2_g"], g["ln2_b"], g["ln4_g"], g["ln4_b"],
                 g["f2_w1"], g["f2_b1"], g["f2_w2"], g["f2_b2"]))):
        wv = aw[2 * E:3 * E]
        bv = ab[2 * E:3 * E]
        W = ow @ wv                      # [E, E]; attn = kv @ W.T + c
        c = ow @ bv + ob                 # [E]
        U = w1 * lna_g[None, :]          # LN1 gain folded into FFN1
        d = b1 + w1 @ lna_b              # LN1 bias folded into FFN1 bias
        V = w2                           # [E, HID]
        e = b2                           # [E]
        arrs[f"w{s}"] = kchunks(W.T, E // P, wdt)
        arrs[f"u{s}"] = kchunks(U.T, E // P, wdt)
        arrs[f"v{s}"] = kchunks(V.T, HID // P, wdt)
        uc = not trivial(c, 0.0)
        ud = not trivial(d, 0.0)
        ue = not trivial(e, 0.0)
        fa = not (trivial(lna_g, 1.0) and trivial(lna_b, 0.0))
        fb = not (trivial(lnb_g, 1.0) and trivial(lnb_b, 0.0))
        if uc:
            arrs[f"c{s}"] = (bscale_c * c).reshape(1, E).astype(BF)
        if ud:
            arrs[f"d{s}"] = (bscale_de * d).reshape(1, HID).astype(BF)
        if ue:
            arrs[f"e{s}"] = (bscale_de * e).reshape(1, E).astype(BF)
        if fa:
            arrs[f"ga{s}"] = lna_g.reshape(1, E).astype(BF)
            arrs[f"ba{s}"] = (bscale_de * lna_b).reshape(1, E).astype(BF)
        if fb:
            arrs[f"gb{s}"] = lnb_g.reshape(1, E).astype(np.float32)
            arrs[f"bb{s}"] = lnb_b.reshape(1, E).astype(np.float32)
        flags.append((uc, ud, ue, fa, fb))

    (uc0, ud0, ue0, fa0, fb0), (uc1, ud1, ue1, fa1, fb1) = flags
    flag_t = (uc0, uc1, ud0, ud1, ue0, ue1, fa0, fa1, fb0, fb1)
    return g, arrs, flag_t


def _pick_rmacro(rows_per_core):
    for r in (512, 256, 128):
        if rows_per_core % r == 0:
            return r
    raise ValueError(f"rows_per_core {rows_per_core} not divisible by 128")


def _core_layouts(x_str, x_tr, sl, NT, R):
    """Per-core straight + transposed DRAM layouts."""
    RC = R // P
    KE = E // P
    xs = np.ascontiguousarray(
        x_str[sl].reshape(NT, RC, P, E).transpose(0, 2, 1, 3))
    xt = np.ascontiguousarray(
        x_tr[sl].reshape(NT, R, KE, P).transpose(0, 3, 2, 1))
    return xs, xt


def prepare(inputs, fp8=USE_FP8):
    """Compile (cached) + build per-core input maps.

    Returns (nc, in_maps, rows_per_core)."""
    g, arrs, flag_t = _prep_host(inputs, fp8)
    B = g["dna"].shape[0]
    rows_per_core = B // NCORES
    rmacro = _pick_rmacro(rows_per_core)
    key = ("v5", fp8, rows_per_core, rmacro, flag_t)
    if key not in _prog_cache:
        _prog_cache[key] = _build_program(rows_per_core, rmacro, flag_t,
                                          fp8)
    nc = _prog_cache[key]

    NT = rows_per_core // rmacro
    xstr = {}
    xtr = {}
    for nm in ("dna", "mol"):
        x = g[nm]
        if fp8:
            xstr[nm] = (64.0 * x).astype(BF)
            xtr[nm] = x.astype(F8)
        else:
            xb = x.astype(BF)
            xstr[nm] = xb
            xtr[nm] = xb
    in_maps = []
    for i in range(NCORES):
        sl = slice(i * rows_per_core, (i + 1) * rows_per_core)
        xs0, xt0 = _core_layouts(xstr["dna"], xtr["dna"], sl, NT, rmacro)
        xs1, xt1 = _core_layouts(xstr["mol"], xtr["mol"], sl, NT, rmacro)
        im = {"xs_dna": xs0, "xt_dna": xt0, "xs_mol": xs1, "xt_mol": xt1}
        im.update(arrs)
        in_maps.append(im)
    return nc, in_maps, rows_per_core


def gather_out(res, rows_per_core):
    outs = []
    for r in res.results:
        o = r["out"]  # [NT, 2, P, RC, E]
        o = o.transpose(0, 3, 2, 1, 4).reshape(rows_per_core, 2 * E)
        outs.append(o)
    return np.concatenate(outs, axis=0).astype(np.float32)


def kernel(**inputs):
    nc, in_maps, rows_per_core = prepare(inputs)
    res = run_bass_kernel_spmd(nc, in_maps, list(range(NCORES)))
    return gather_out(res, rows_per_core)
